# revision 6
# baseline (speedup 1.0000x reference)
"""Trainium2 Bass kernel for nn_AtomToFactor (GNN message passing).

Computation per factor type (bond k=2, angle k=3, torsion k=4):
    msgs = atom_feat[idx]                      # [N, k, 64] gather
    x_f  = concat(msgs.reshape(N, k*64), cur)  # [N, k*64+1]
    out  = MLP(x_f) + MLP(x_r)                 # x_r = slot-reversed msgs

Strategy (8 cores, data-parallel over factors; atom table + weights replicated):
  - fp16 atom table in DRAM; rows gathered with batched gpsimd indirect DMA
    (int32 indices, one 128-partition x GROUP-column gather per slot-group).
  - Gathered [factor, feat] tiles are flipped to [feat, factor] with PE
    transpose-mode; fwd+rev symmetrization is folded into stacked weights:
    L1 lhsT has fwd HID in cols 0-63 and rev HID in cols 64-127, L2 is
    block-diag(W2, W2), L3 is vstack(W3, W3) which sums both branches.
  - Outputs are produced transposed [10, N] on device and rearranged on host.
"""

import numpy as np

P = 128
D = 64
HID = 64
OUT = 10
N_ATOMS = 100000
N_CORES = 8
GROUP = 32   # gather-group size in idx-plane columns (chunks of 128 factors)
TILE_CH = 4  # chunks per MLP tile -> N=512 matmuls

# (short name, input prefix, k, total factor count)
FTYPES = (
    ("b", "bond", 2, 200000),
    ("a", "angle", 3, 300000),
    ("t", "torsion", 4, 500000),
)


def _ceil_to(x, m):
    return -(-x // m) * m


def type_cfg(n_total, n_cores=N_CORES):
    """Per-core factor count and padded idx-plane column count."""
    n_c = -(-n_total // n_cores)
    c_cols = _ceil_to(-(-n_c // P), TILE_CH)
    return n_c, c_cols


# --------------------------------------------------------------------------
# Host-side packing
# --------------------------------------------------------------------------

def pack_weights(W1, b1, W2, b2, W3, b3, k):
    """Pack MLP params into the stacked fwd/rev stationary layouts."""
    W1 = np.asarray(W1, np.float32)
    blocks = [W1[64 * s:64 * (s + 1)] for s in range(k)]   # each [64, HID]
    w1cur = W1[64 * k:64 * k + 1]                          # [1, HID]

    w1a = np.zeros((P, P), np.float32)
    w1a[0:64, 0:64] = blocks[0]
    w1a[64:128, 0:64] = blocks[1]
    w1a[0:64, 64:128] = blocks[k - 1]
    w1a[64:128, 64:128] = blocks[k - 2]

    if k == 2:
        w1b = None
    elif k == 3:
        w1b = np.zeros((64, P), np.float32)
        w1b[:, 0:64] = blocks[2]
        w1b[:, 64:128] = blocks[0]
    else:  # k == 4
        w1b = np.zeros((P, P), np.float32)
        w1b[0:64, 0:64] = blocks[2]
        w1b[64:128, 0:64] = blocks[3]
        w1b[0:64, 64:128] = blocks[1]
        w1b[64:128, 64:128] = blocks[0]

    w1r = np.concatenate([w1cur, w1cur], axis=1)           # [1, 128]

    w2bd = np.zeros((P, P), np.float32)
    w2bd[0:64, 0:64] = W2
    w2bd[64:128, 64:128] = W2

    w3st = np.concatenate([W3, W3], axis=0)                # [128, OUT]

    b1st = np.concatenate([b1, b1]).reshape(P, 1).astype(np.float32)
    b2st = np.concatenate([b2, b2]).reshape(P, 1).astype(np.float32)
    b3x2 = (2.0 * np.asarray(b3)).reshape(OUT, 1).astype(np.float32)

    f16 = np.float16
    out = {
        "w1a": w1a.astype(f16), "w1r": w1r.astype(f16),
        "w2": w2bd.astype(f16), "w3": w3st.astype(f16),
        "b1": b1st, "b2": b2st, "b3": b3x2,
    }
    if w1b is not None:
        out["w1b"] = w1b.astype(f16)
    return out


def gather_units(k):
    """Gather units per type: ('pair', (s0, s1)) or ('solo', s)."""
    if k == 2:
        return [("pair", (0, 1))]
    if k == 3:
        return [("pair", (0, 1)), ("solo", (2,))]
    return [("pair", (0, 1)), ("pair", (2, 3))]


def pack_idx_repr(idx, rep, c_cols):
    """Shard-local index planes [128, k*C] and device-order repr [1, 128*C].

    Factor f = p*C + c lives on partition p, idx-plane column c; gather chunk c
    emits device output columns c*128+p. Slot pairs are interleaved
    (col 2c+sl) so a single indirect DMA fills a [128, csz, 2, 64] pair-tile.
    """
    n_c, k = idx.shape
    npad = P * c_cols
    idx_pad = np.zeros((npad, k), np.int32)
    idx_pad[:n_c] = idx.astype(np.int32)
    planes = idx_pad.reshape(P, c_cols, k)  # [p, c, s]
    blocks = []
    for kind, ss in gather_units(k):
        if kind == "pair":
            blocks.append(planes[:, :, list(ss)].reshape(P, 2 * c_cols))
        else:
            blocks.append(planes[:, :, ss[0]])
    idx_plane = np.ascontiguousarray(np.concatenate(blocks, axis=1))

    rep_pad = np.zeros((npad,), np.float32)
    rep_pad[:n_c] = np.asarray(rep).reshape(-1)
    # repr_dev[c*128 + p] = rep_pad[p*C + c]
    rep_dev = rep_pad.reshape(P, c_cols).transpose(1, 0).reshape(1, npad)
    return idx_plane, rep_dev.astype(np.float16)


def unpack_out(dev_out, c_cols, n_c):
    """Device [10, 128*C] (col c*128+p = factor p*C+c) -> [n_c, 10]."""
    a = dev_out.reshape(OUT, c_cols, P).transpose(0, 2, 1).reshape(OUT, P * c_cols)
    return np.ascontiguousarray(a[:, :n_c].T)


# --------------------------------------------------------------------------
# Bass program
# --------------------------------------------------------------------------

def build_program(natoms, tcfgs):
    """tcfgs: list of dicts(nm, k, C). Returns the assembled Bass module."""
    import concourse.bacc as bacc
    import concourse.bass as bass
    import concourse.tile as tile
    from concourse import mybir
    from concourse.bass import IndirectOffsetOnAxis

    f16 = mybir.dt.float16
    f32 = mybir.dt.float32
    i32 = mybir.dt.int32
    AF = mybir.ActivationFunctionType

    nc = bacc.Bacc("TRN2", debug=False, target_bir_lowering=False)

    table = nc.dram_tensor("table", [natoms, D], f16, kind="ExternalInput").ap()
    ident = nc.dram_tensor("ident", [P, P], f16, kind="ExternalInput").ap()
    dram = {}
    for t in tcfgs:
        nm, k, C = t["nm"], t["k"], t["C"]
        dram[nm, "idx"] = nc.dram_tensor(f"{nm}_idx", [P, k * C], i32, kind="ExternalInput").ap()
        dram[nm, "repr"] = nc.dram_tensor(f"{nm}_repr", [1, P * C], f16, kind="ExternalInput").ap()
        for w, shp, dt in (("w1a", [P, P], f16), ("w1r", [1, P], f16),
                           ("w2", [P, P], f16), ("w3", [P, OUT], f16),
                           ("b1", [P, 1], f32), ("b2", [P, 1], f32), ("b3", [OUT, 1], f32)):
            dram[nm, w] = nc.dram_tensor(f"{nm}_{w}", shp, dt, kind="ExternalInput").ap()
        if k == 3:
            dram[nm, "w1b"] = nc.dram_tensor(f"{nm}_w1b", [64, P], f16, kind="ExternalInput").ap()
        elif k == 4:
            dram[nm, "w1b"] = nc.dram_tensor(f"{nm}_w1b", [P, P], f16, kind="ExternalInput").ap()
        dram[nm, "out"] = nc.dram_tensor(f"{nm}_out", [OUT, P * C], f32, kind="ExternalOutput").ap()

    with tile.TileContext(nc) as tc:
        with (
            tc.tile_pool(name="const", bufs=1) as cpool,
            tc.tile_pool(name="gtiles", bufs=2) as gpool,
            tc.tile_pool(name="rhs", bufs=3) as rhspool,
            tc.tile_pool(name="hact", bufs=3) as hpool,
            tc.tile_pool(name="osb", bufs=2) as opool,
            tc.tile_pool(name="reprp", bufs=2) as rpool,
            tc.tile_pool(name="ptr", bufs=2, space="PSUM") as ptrpool,
            tc.tile_pool(name="pmm", bufs=2, space="PSUM") as pmmpool,
            tc.tile_pool(name="pout", bufs=2, space="PSUM") as poutpool,
        ):
            ident_sb = cpool.tile([P, P], f16, tag="ident")
            nc.sync.dma_start(out=ident_sb[:], in_=ident)

            # per-type constants
            wsb = {}
            for t in tcfgs:
                nm, k = t["nm"], t["k"]
                for w in ("w1a", "w1r", "w2", "w3", "b1", "b2", "b3"):
                    ap = dram[nm, w]
                    sb = cpool.tile(list(ap.shape), ap.dtype, tag=f"{nm}_{w}")
                    nc.sync.dma_start(out=sb[:], in_=ap)
                    wsb[nm, w] = sb
                if k >= 3:
                    ap = dram[nm, "w1b"]
                    sb = cpool.tile(list(ap.shape), ap.dtype, tag=f"{nm}_w1b")
                    nc.sync.dma_start(out=sb[:], in_=ap)
                    wsb[nm, "w1b"] = sb

            for t in tcfgs:
                nm, k, C = t["nm"], t["k"], t["C"]
                pairs = [(0, 1)] if k < 4 else [(0, 1), (2, 3)]
                solo = 2 if k == 3 else None

                idx_sb = cpool.tile([P, k * C], i32, tag=f"{nm}_idx")
                nc.sync.dma_start(out=idx_sb[:], in_=dram[nm, "idx"])

                units = gather_units(k)
                # column offset of each unit's block inside the idx plane
                uoffs = []
                off = 0
                for kind, ss in units:
                    uoffs.append(off)
                    off += 2 * C if kind == "pair" else C

                for c0 in range(0, C, GROUP):
                    csz = min(GROUP, C - c0)
                    gps = []
                    gso = None
                    for (kind, ss), uoff in zip(units, uoffs):
                        if kind == "pair":
                            g = gpool.tile([P, GROUP, 2 * D], f16, tag=f"g{len(gps)}")
                            gps.append(g)
                            nc.gpsimd.indirect_dma_start(
                                out=g[:, 0:csz, :],
                                out_offset=None,
                                in_=table,
                                in_offset=IndirectOffsetOnAxis(
                                    ap=idx_sb[:, uoff + 2 * c0:uoff + 2 * (c0 + csz)],
                                    axis=0,
                                ),
                            )
                        else:
                            gso = gpool.tile([P, GROUP, D], f16, tag="gsolo")
                            nc.gpsimd.indirect_dma_start(
                                out=gso[:, 0:csz, :],
                                out_offset=None,
                                in_=table,
                                in_offset=IndirectOffsetOnAxis(
                                    ap=idx_sb[:, uoff + c0:uoff + c0 + csz],
                                    axis=0,
                                ),
                            )

                    rp = rpool.tile([1, GROUP * P], f16, tag="repr")
                    nc.sync.dma_start(out=rp[:, 0:csz * P],
                                      in_=dram[nm, "repr"][:, c0 * P:(c0 + csz) * P])
                    osb = opool.tile([OUT, GROUP * P], f32, tag="osb")

                    for tl in range(csz // TILE_CH):
                        ptr = ptrpool.tile([P, 1024], f16, tag="ptr")
                        for j4 in range(TILE_CH):
                            j = tl * TILE_CH + j4
                            for pi in range(len(pairs)):
                                nc.tensor.transpose(
                                    out=ptr[:, pi * 512 + j4 * P:pi * 512 + (j4 + 1) * P],
                                    in_=gps[pi][:, j, :],
                                    identity=ident_sb[:],
                                )
                            if solo is not None:
                                nc.tensor.transpose(
                                    out=ptr[0:D, 512 + j4 * P:512 + (j4 + 1) * P],
                                    in_=gso[:, j, :],
                                    identity=ident_sb[:],
                                )

                        rhsa = rhspool.tile([P, 512], f16, tag="rhsa")
                        nc.vector.tensor_copy(out=rhsa[:], in_=ptr[:, 0:512])
                        rhsb = None
                        if k >= 3:
                            kb = 64 if k == 3 else P
                            rhsb = rhspool.tile([P, 512], f16, tag="rhsb")
                            nc.vector.tensor_copy(out=rhsb[0:kb, :], in_=ptr[0:kb, 512:1024])

                        p1 = pmmpool.tile([P, 512], f32, tag="p1")
                        nc.tensor.matmul(out=p1[:], lhsT=wsb[nm, "w1a"][:], rhs=rhsa[:],
                                         start=True, stop=False)
                        if k >= 3:
                            kb = 64 if k == 3 else P
                            nc.tensor.matmul(out=p1[:], lhsT=wsb[nm, "w1b"][0:kb, :],
                                             rhs=rhsb[0:kb, :], start=False, stop=False)
                        nc.tensor.matmul(out=p1[:], lhsT=wsb[nm, "w1r"][:],
                                         rhs=rp[:, tl * 512:(tl + 1) * 512],
                                         start=False, stop=True)

                        h1 = hpool.tile([P, 512], f16, tag="h1")
                        nc.scalar.activation(out=h1[:], in_=p1[:], func=AF.Relu,
                                             bias=wsb[nm, "b1"][:, 0:1])
                        p2 = pmmpool.tile([P, 512], f32, tag="p2")
                        nc.tensor.matmul(out=p2[:], lhsT=wsb[nm, "w2"][:], rhs=h1[:],
                                         start=True, stop=True)
                        h2 = hpool.tile([P, 512], f16, tag="h2")
                        nc.scalar.activation(out=h2[:], in_=p2[:], func=AF.Relu,
                                             bias=wsb[nm, "b2"][:, 0:1])
                        p3 = poutpool.tile([OUT, 512], f32, tag="p3")
                        nc.tensor.matmul(out=p3[:], lhsT=wsb[nm, "w3"][:], rhs=h2[:],
                                         start=True, stop=True)
                        nc.scalar.activation(out=osb[:, tl * 512:(tl + 1) * 512],
                                             in_=p3[:], func=AF.Identity,
                                             bias=wsb[nm, "b3"][:, 0:1])

                    nc.sync.dma_start(out=dram[nm, "out"][:, c0 * P:(c0 + csz) * P],
                                      in_=osb[:, 0:csz * P])
    nc.compile()
    return nc


# --------------------------------------------------------------------------
# Entry point
# --------------------------------------------------------------------------

def _make_in_maps(inputs, tcfgs, n_cores=N_CORES):
    table16 = np.ascontiguousarray(np.asarray(inputs["atom_feat"], np.float32)).astype(np.float16)
    ident = np.eye(P, dtype=np.float16)

    shared = {"table": table16, "ident": ident}
    for (nm, pref, k, n_total), t in zip(FTYPES, tcfgs):
        w = pack_weights(
            inputs[nm + "W1"], inputs[nm + "b1"],
            inputs[nm + "W2"], inputs[nm + "b2"],
            inputs[nm + "W3"], inputs[nm + "b3"], k)
        for key, val in w.items():
            shared[f"{nm}_{key}"] = np.ascontiguousarray(val)

    in_maps = []
    for m in range(n_cores):
        im = dict(shared)
        for (nm, pref, k, n_total), t in zip(FTYPES, tcfgs):
            n_c, C = t["n_c"], t["C"]
            idx = np.asarray(inputs[pref + "_idx"])[m * n_c:(m + 1) * n_c]
            rep = np.asarray(inputs[pref + "_repr"], np.float32)[m * n_c:(m + 1) * n_c]
            idx_plane, rep_dev = pack_idx_repr(idx, rep, C)
            im[f"{nm}_idx"] = idx_plane
            im[f"{nm}_repr"] = rep_dev
        in_maps.append(im)
    return in_maps


_CACHE = {}
TRACE = False  # set True (e.g. from test.py) to capture an NTFF profile


def kernel(**inputs):
    from concourse.bass_utils import run_bass_kernel_spmd

    tcfgs = []
    for nm, pref, k, n_total in FTYPES:
        n_c, C = type_cfg(n_total)
        tcfgs.append({"nm": nm, "k": k, "C": C, "n_c": n_c})

    if "nc" not in _CACHE:
        _CACHE["nc"] = build_program(N_ATOMS, tcfgs)
    nc = _CACHE["nc"]

    in_maps = _make_in_maps(inputs, tcfgs)
    res = run_bass_kernel_spmd(nc, in_maps, core_ids=list(range(N_CORES)),
                               trace=TRACE)
    _CACHE["last_res"] = res

    outs = []
    for nm, pref, k, n_total in FTYPES:
        t = next(c for c in tcfgs if c["nm"] == nm)
        parts = [unpack_out(res.results[m][f"{nm}_out"], t["C"], t["n_c"])
                 for m in range(N_CORES)]
        outs.append(np.concatenate(parts, axis=0)[:n_total])
    return tuple(outs)


if __name__ == "__main__":
    # smoke-test the host packing round trip
    rng = np.random.default_rng(0)
    for nm, pref, k, n_total in FTYPES:
        n_c, C = type_cfg(n_total)
        idx = rng.integers(0, N_ATOMS, (n_c, k))
        rep = rng.standard_normal((n_c, 1)).astype(np.float32)
        ip, rd = pack_idx_repr(idx, rep, C)
        # factor f = p*C+c -> plane[p, s*C+c]
        f = 12345 % n_c
        p, c = f // C, f % C
        assert ip[p, 0 * C + c] == idx[f, 0]
        assert rd[0, c * P + p] == np.float16(rep[f, 0])
        dev = rng.standard_normal((OUT, P * C)).astype(np.float32)
        up = unpack_out(dev, C, n_c)
        assert up[f, 3] == dev[3, c * P + p]
    print("host packing ok")
